# revision 19
# baseline (speedup 1.0000x reference)
"""ChannelAttention (Softmax2d-over-batch) Trainium2 kernel, 8-core SPMD.

v5: single fused pipeline; PE is kept continuously busy and the batch
AllReduce of S = sum_b exp(scores) is split into 3 chunks on separate
DRAM tensors, issued mid-stream so they land under the V GEMMs.

Structure (per core, 4 samples):
  kt:   Kt[b] = ((Wk @ x_b)^T + bk) -> SBUF bf16 [HW, C]    (PE 47us)
  qt0:  Qt cgroup0 (cols 0:512)                              (PE 14us)
  B:    for dt 0..9: scoresT[d,c] = Qt_dt^T Kt (both hwt)   (PE 43us)
          -> ACT exp -> E bf16; S[dt] = sum_b E_b[dt] (DVE tree)
        qt cgroups 1,2 interleaved into dt 0..5 so ACT exp
        (66us, the B bottleneck) overlaps PE qt work.
        AR chunks (gpsimd->TOPSP, Pool does nothing else):
          dt2 -> AR0(S[0:3]), dt5 -> AR1(S[3:6]), dt9 -> AR2(S[6:10])
  C1:   V GEMMs (PE 43us, N=512 over sample pairs) hide the ARs;
        per-dt: z=AR out -> f32 (ACT) -> 1/z (DVE) -> E*=R in place
        (DVE/GpSimd split), pipelined behind the AR chunk landings.
  C2:   att[b] = attnT-contract @ V[b] (PE 43us), psum->SBUF on ACT,
        att stored [P, ct, b, HW] so refine streams N=512.
  C3:   refine GEMMs (PE 43us) interleaved with C2 per sample-pair;
        out = alpha*psum + (alpha*br + x_bf16), stored per (b, ot).

SBUF plan (strict LIFO per side; ~204 KB/p peak):
  left:  cpool | xb 20K (whole kernel) | ktqt 40K (..B) | wk 25.6K (..kt)
         then sst 7.5K (B) | then wr 25.6K, o 3K, att 20K, wv 5K (C)
  right: (after kt) E 102.4K (B..att) | wq 25.6K (..dt5)
         then V 20K, zb 2.5K, r 5K (C)
"""

import os

import numpy as np
import ml_dtypes

import concourse.bass as bass
import concourse.tile as tile
from concourse import bacc, mybir
from concourse import bass_utils

B, C, S, HW = 32, 1280, 16, 256
P = 128
KC = C // P          # 10 chunks of the channel dim
NCORES = 8
BL = B // NCORES     # 4 samples per core
SHIFT = 45.0
CGROUPS = [(0, 512), (512, 512), (1024, 256)]  # psum-bank-sized col groups
F32 = mybir.dt.float32
BF16 = mybir.dt.bfloat16
AF = mybir.ActivationFunctionType

_CACHE = {}
# AllReduce chunking: list of (start_dt, n_dt); chunk issued after its
# last dt's S row is stored.
AR_MODE = os.environ.get("KERNEL_AR_MODE", "split3")
# phase truncation for differential timing: 1=kt 2=+qt0 3=+B 4=+V/recip 5=+att 6=full
PHASES = int(os.environ.get("KERNEL_PHASES", "6"))
SCHUNKS = {
    "split3": [(0, 3), (3, 3), (6, 4)],
    "split2": [(0, 5), (5, 5)],
    "single": [(0, 10)],
    "none": [(0, 3), (3, 3), (6, 4)],  # diagnostic: DMA copy, no collective
}[AR_MODE]


def _emit(nc, tc, io, alpha):
    ones, bvc, brc = io["ones_t"], io["bvc_t"], io["brc_t"]
    xb_d = io["xb_d"]
    wk_d, wq_d, wv_d, wr_d = io["wk_d"], io["wq_d"], io["wv_d"], io["wr_d"]
    out_d = io["out_d"]

    # ---------------- pools: left stack base ----------------
    xbp_ctx = tc.tile_pool(name="xbp", bufs=1, side="left")
    xbp = xbp_ctx.__enter__()
    xb = xbp.tile([P, KC, BL * HW], BF16, tag="xb")    # 20 KB/p, whole kernel
    kt_ctx = tc.tile_pool(name="ktp", bufs=1, side="left")
    ktp = kt_ctx.__enter__()
    kt = ktp.tile([P, 2, BL, C], BF16, tag="kt")       # 20 KB/p
    wk_ctx = tc.tile_pool(name="wkp", bufs=1, side="left")
    wkp = wk_ctx.__enter__()
    wk_sb = wkp.tile([P, KC, C], BF16, tag="wk")       # 25.6 KB/p
    brow_k = wkp.tile([1, C], BF16, tag="browk")
    # E + wq live on the right stack from the start so the wq load does
    # not land on (and wait for) the released wk zone.
    ep_ctx = tc.tile_pool(name="ep", bufs=1, side="right")
    ep = ep_ctx.__enter__()
    e_sb = ep.tile([P, BL, KC, C], BF16, tag="E")      # 102.4 KB/p
    wq_ctx = tc.tile_pool(name="wqp", bufs=1, side="right")
    wqp = wq_ctx.__enter__()
    wq_sb = wqp.tile([P, KC, C], BF16, tag="wq")       # 25.6 KB/p
    brow_q = wqp.tile([1, C], BF16, tag="browq")

    # load order on SP: wk, bias rows, xb per-sample (b0 first), wq
    nc.sync.dma_start(wk_sb[:], wk_d.ap().rearrange("(k p) n -> p k n", p=P))
    nc.sync.dma_start(brow_k[:], io["bk"].ap())
    for b in range(BL):
        nc.sync.dma_start(
            xb[:, :, b * HW:(b + 1) * HW],
            xb_d.ap()[b].rearrange("(k p) n -> p k n", p=P),
        )
    nc.sync.dma_start(wq_sb[:], wq_d.ap().rearrange("(k p) n -> p k n", p=P))
    nc.sync.dma_start(brow_q[:], io["bq"].ap())

    def proj_group(dest, w_sb, brow, b, hwt, psp):
        """dest[:, hwt, b, :] = (x_b^T W)[hw-chunk, :] + bias.

        k-outer with one psum bank per cgroup: each xb[k] weight load
        feeds 3 consecutive matmuls (dup LDWEIGHTS removed post-sched).
        """
        ps = psp.tile([P, C], F32, tag="psA")  # 2.5 banks, bank-aligned slices
        for k in range(KC):
            for cgs, cgl in CGROUPS:
                nc.tensor.matmul(
                    ps[:, cgs:cgs + cgl],
                    xb[:, k, b * HW + hwt * P:b * HW + (hwt + 1) * P],
                    w_sb[:, k, cgs:cgs + cgl],
                    start=(k == 0),
                    stop=False,
                )
        for cgs, cgl in CGROUPS:
            nc.tensor.matmul(
                ps[:, cgs:cgs + cgl], ones[:, :P], brow[:, cgs:cgs + cgl],
                start=False, stop=True,
            )
        # single DVE drain; keeps ACT free for the exp stream in B
        nc.vector.tensor_copy(dest[:, hwt, b, :], ps[:])

    # ---------------- kt (whole) ----------------
    psA_ctx = tc.tile_pool(name="psA", bufs=2, space="PSUM", side="left")
    psA = psA_ctx.__enter__()
    for b in range(BL):
        for hwt in range(2):
            proj_group(kt, wk_sb, brow_k, b, hwt, psA)
    wk_ctx.__exit__(None, None, None)
    if PHASES <= 1:
        psA_ctx.__exit__(None, None, None)
        wq_ctx.__exit__(None, None, None)
        ep_ctx.__exit__(None, None, None)
        kt_ctx.__exit__(None, None, None)
        xbp_ctx.__exit__(None, None, None)
        return
    qt_ctx = tc.tile_pool(name="qtp", bufs=1, side="left")
    qtp = qt_ctx.__enter__()
    qt = qtp.tile([P, 2, BL, C], BF16, tag="qt")       # 20 KB/p

    # ---------------- qt (whole) ----------------
    for b in range(BL):
        for hwt in range(2):
            proj_group(qt, wq_sb, brow_q, b, hwt, psA)
    psA_ctx.__exit__(None, None, None)
    wq_ctx.__exit__(None, None, None)

    if PHASES <= 2:
        qt_ctx.__exit__(None, None, None)
        ep_ctx.__exit__(None, None, None)
        kt_ctx.__exit__(None, None, None)
        xbp_ctx.__exit__(None, None, None)
        return

    # which AR chunk owns dt, and the chunk-local row index
    dt2chunk = {}
    for ci, (d0, nd) in enumerate(SCHUNKS):
        for j in range(nd):
            dt2chunk[d0 + j] = (ci, j)

    # ---------------- fused B ----------------
    sst_ctx = tc.tile_pool(name="sst", bufs=1, side="left")
    sstp = sst_ctx.__enter__()                          # 12.5 KB/p
    # first two V-weight chunks prefetched during B so the V GEMMs can
    # start the instant the last scores matmul retires
    wv01 = sstp.tile([P, KC, 2 * P], BF16, tag="wv01")
    nc.sync.dma_start(
        wv01[:],
        wv_d.ap()[:, 0:2 * P].rearrange("(k p) n -> p k n", p=P),
    )
    psB_ctx = tc.tile_pool(name="psB", bufs=2, space="PSUM", side="right")
    psB = psB_ctx.__enter__()
    for dt in range(KC):
        for b in range(BL):
            ps = psB.tile([P, C], F32, tag="psB")
            # hwt-outer: each qt chunk load feeds 3 consecutive matmuls
            for hwt in range(2):
                for cgs, cgl in CGROUPS:
                    nc.tensor.matmul(
                        ps[:, cgs:cgs + cgl],
                        qt[:, hwt, b, dt * P:(dt + 1) * P],
                        kt[:, hwt, b, cgs:cgs + cgl],
                        start=(hwt == 0),
                        stop=(hwt == 1),
                    )
            nc.scalar.activation(
                e_sb[:, b, dt, :], ps[:], AF.Exp, bias=-SHIFT, scale=1.0,
            )
        # S[dt] = (E0+E1) + (E2+E3), bf16 pairwise tree on DVE
        s01 = sstp.tile([P, C], BF16, tag="s01")
        s23 = sstp.tile([P, C], BF16, tag="s23")
        st = sstp.tile([P, C], BF16, tag="st")
        nc.vector.tensor_add(s01[:], e_sb[:, 0, dt], e_sb[:, 1, dt])
        nc.vector.tensor_add(s23[:], e_sb[:, 2, dt], e_sb[:, 3, dt])
        nc.vector.tensor_add(st[:], s01[:], s23[:])
        ci, j = dt2chunk[dt]
        nc.sync.dma_start(io["s_in"][ci].ap()[j], st[:])
        # AR chunk issues (gpsimd queue holds only the 3 collectives)
        for cix, (d0, nd) in enumerate(SCHUNKS):
            if dt == d0 + nd - 1:
                if AR_MODE == "none":
                    # diagnostic only: wrong result, measures non-AR path
                    nc.gpsimd.dma_start(
                        io["s_out"][cix].ap(), io["s_in"][cix].ap()
                    )
                else:
                    nc.gpsimd.collective_compute(
                        "AllReduce",
                        mybir.AluOpType.add,
                        replica_groups=[list(range(NCORES))],
                        ins=[io["s_in"][cix].ap()],
                        outs=[io["s_out"][cix].ap()],
                    )
    if PHASES <= 3:
        psB_ctx.__exit__(None, None, None)
        sst_ctx.__exit__(None, None, None)
        qt_ctx.__exit__(None, None, None)
        kt_ctx.__exit__(None, None, None)
        ep_ctx.__exit__(None, None, None)
        xbp_ctx.__exit__(None, None, None)
        return

    # ---------------- C1: V GEMMs start immediately at B end ----------
    vp_ctx = tc.tile_pool(name="vp", bufs=1, side="right")
    vp = vp_ctx.__enter__()
    v_sb = vp.tile([P, KC, BL * HW], BF16, tag="V")     # 20 KB/p
    psV_ctx = tc.tile_pool(name="psV", bufs=1, space="PSUM", side="left")
    psV = psV_ctx.__enter__()

    # wv chunk loads (per vct) + wr load early on SP
    def recip_and_muls(dt):
        """z[dt] -> f32 -> 1/z -> E[:, :, dt, :] *= R  (in place)."""
        ci, j = dt2chunk[dt]
        zb = zbp.tile([P, C], BF16, tag="zb")
        # ACT-issued: keeps the AR-gated z loads out of SP's FIFO (the
        # wv/wr loads behind them must not wait on the collectives)
        nc.scalar.dma_start(zb[:], io["s_out"][ci].ap()[j])
        r = rp.tile([P, C], BF16, tag="r")
        for cgs, cgl in CGROUPS:
            # NR step may read only one non-scalar input from PSUM:
            # z lives in SBUF f32, scratch + result in PSUM.
            zf = zfp.tile([P, 512], F32, tag="zf")
            scr = psR.tile([P, 512], F32, tag="scr")
            rf = psR.tile([P, 512], F32, tag="rf")
            nc.scalar.copy(zf[:, :cgl], zb[:, cgs:cgs + cgl])
            nc.vector.reciprocal_approx_accurate(
                rf[:, :cgl], zf[:, :cgl], scr[:, :cgl]
            )
            nc.scalar.copy(r[:, cgs:cgs + cgl], rf[:, :cgl])
        for b in range(BL):
            eng = nc.gpsimd if b % 2 == 1 else nc.vector
            eng.tensor_mul(e_sb[:, b, dt], e_sb[:, b, dt], r[:])

    def v_gemms(vct, wvt, col=0):
        ps0 = psV.tile([P, 512], F32, tag="psV0")
        ps1 = psV.tile([P, 512], F32, tag="psV1")
        for ci_ in range(KC):
            # one wv chunk load feeds both sample-pair matmuls
            for bp, ps in ((0, ps0), (1, ps1)):
                nc.tensor.matmul(
                    ps[:],
                    wvt[:, ci_, col:col + P],
                    xb[:, ci_, bp * 512:(bp + 1) * 512],
                    start=(ci_ == 0),
                    stop=(ci_ == KC - 1),
                )
        for bp, ps in ((0, ps0), (1, ps1)):
            nc.vector.tensor_scalar_add(
                v_sb[:, vct, bp * 512:(bp + 1) * 512], ps[:],
                bvc[:, vct:vct + 1],
            )

    # vct 0,1 straight from the prefetched wv01 (B pools still open)
    v_gemms(0, wv01, col=0)
    v_gemms(1, wv01, col=P)
    psB_ctx.__exit__(None, None, None)
    sst_ctx.__exit__(None, None, None)
    qt_ctx.__exit__(None, None, None)
    kt_ctx.__exit__(None, None, None)

    # ---------------- remaining C pools ----------------
    zb_ctx = tc.tile_pool(name="zbp", bufs=1, side="right")
    zbp = zb_ctx.__enter__()                            # 2.5 KB/p
    rp_ctx = tc.tile_pool(name="rp", bufs=2, side="right")
    rp = rp_ctx.__enter__()                             # 5 KB/p
    zf_ctx = tc.tile_pool(name="zfp", bufs=2, side="right")
    zfp = zf_ctx.__enter__()                            # 4 KB/p
    wr_ctx = tc.tile_pool(name="wrp", bufs=1, side="left")
    wrp = wr_ctx.__enter__()
    wr_sb = wrp.tile([P, KC, C], BF16, tag="wr")        # 25.6 KB/p
    o_ctx = tc.tile_pool(name="op", bufs=1, side="left")
    op = o_ctx.__enter__()                              # 4 KB/p
    att_ctx = tc.tile_pool(name="attp", bufs=1, side="left")
    attp = att_ctx.__enter__()
    att_sb = attp.tile([P, KC, BL * HW], BF16, tag="att")  # 20 KB/p
    wv_ctx = tc.tile_pool(name="wvp", bufs=2, side="left")
    wvp = wv_ctx.__enter__()                            # 5 KB/p
    psR_ctx = tc.tile_pool(name="psR", bufs=1, space="PSUM", side="right")
    psR = psR_ctx.__enter__()

    def wv_load(vct):
        t = wvp.tile([P, KC, P], BF16, tag="wvc", name=f"wv{vct}")
        nc.sync.dma_start(
            t[:],
            wv_d.ap()[:, vct * P:(vct + 1) * P].rearrange(
                "(k p) n -> p k n", p=P),
        )
        return t

    # V GEMMs with recip/mul chains pipelined behind the AR landings
    recip_and_muls(0)
    recip_and_muls(1)
    wv_next = wv_load(2)
    wr_loaded = False
    for vct in range(2, KC):
        wvt = wv_next
        if vct < KC - 1:
            wv_next = wv_load(vct + 1)
        v_gemms(vct, wvt)
        if vct <= 5:
            recip_and_muls(vct)
        if vct == 3 and not wr_loaded:
            nc.sync.dma_start(
                wr_sb[:], wr_d.ap().rearrange("(k p) n -> p k n", p=P))
            wr_loaded = True
    for dt in range(6, KC):
        recip_and_muls(dt)
    wv_ctx.__exit__(None, None, None)
    psV_ctx.__exit__(None, None, None)
    if PHASES <= 4:
        psR_ctx.__exit__(None, None, None)
        att_ctx.__exit__(None, None, None)
        o_ctx.__exit__(None, None, None)
        wr_ctx.__exit__(None, None, None)
        zf_ctx.__exit__(None, None, None)
        rp_ctx.__exit__(None, None, None)
        zb_ctx.__exit__(None, None, None)
        vp_ctx.__exit__(None, None, None)
        ep_ctx.__exit__(None, None, None)
        xbp_ctx.__exit__(None, None, None)
        return

    # ---------------- C2/C3: att + refine, interleaved ----------------
    attps_ctx = tc.tile_pool(name="attps", bufs=3, space="PSUM", side="left")
    attps = attps_ctx.__enter__()

    def att_gemms(b):
        for ct in range(KC):
            ps = attps.tile([P, HW], F32, tag="psAtt")
            for dt in range(KC):
                nc.tensor.matmul(
                    ps[:],
                    e_sb[:, b, dt, ct * P:(ct + 1) * P],
                    v_sb[:, dt, b * HW:(b + 1) * HW],
                    start=(dt == 0),
                    stop=(dt == KC - 1),
                )
            nc.scalar.copy(att_sb[:, ct, b * HW:(b + 1) * HW], ps[:])

    out_ap = out_d.ap().rearrange("b (k p) n -> p k b n", p=P)

    def refine_all(refps):
        oo = [None, None]
        for ot in range(KC):
            ps0 = refps.tile([P, 512], F32, tag="psRef0")
            ps1 = refps.tile([P, 512], F32, tag="psRef1")
            for ct in range(KC):
                # one wr chunk load feeds both sample-pair matmuls
                for bp, ps in ((0, ps0), (1, ps1)):
                    nc.tensor.matmul(
                        ps[:],
                        wr_sb[:, ct, ot * P:(ot + 1) * P],
                        att_sb[:, ct, bp * 512:(bp + 1) * 512],
                        start=(ct == 0),
                        stop=(ct == KC - 1),
                    )
            if ot % 2 == 0:
                oo = [op.tile([P, 2, 2, HW], BF16, tag=f"o{bp}",
                              name=f"oo{bp}_{ot}")
                      for bp in range(2)]
            for bp, ps in ((0, ps0), (1, ps1)):
                for j in range(2):
                    b = 2 * bp + j
                    # out = alpha * psum + (alpha*br + x), bf16 store
                    nc.vector.affine_then_add(
                        oo[bp][:, ot % 2, j, :], ps[:, j * HW:(j + 1) * HW],
                        xb[:, ot, b * HW:(b + 1) * HW],
                        scale=alpha, bias=brc[:, ot:ot + 1],
                    )
            if ot % 2 == 1:
                for bp in range(2):
                    for j in range(2):
                        nc.sync.dma_start(
                            out_ap[:, ot - 1:ot + 1, 2 * bp + j, :],
                            oo[bp][:, :, j, :],
                        )

    if PHASES <= 5:
        att_gemms(0)
        att_gemms(1)
        att_gemms(2)
        att_gemms(3)
        attps_ctx.__exit__(None, None, None)
        psR_ctx.__exit__(None, None, None)
        att_ctx.__exit__(None, None, None)
        o_ctx.__exit__(None, None, None)
        wr_ctx.__exit__(None, None, None)
        zf_ctx.__exit__(None, None, None)
        rp_ctx.__exit__(None, None, None)
        zb_ctx.__exit__(None, None, None)
        vp_ctx.__exit__(None, None, None)
        ep_ctx.__exit__(None, None, None)
        xbp_ctx.__exit__(None, None, None)
        return
    att_gemms(0)
    att_gemms(1)
    att_gemms(2)
    att_gemms(3)
    psR_ctx.__exit__(None, None, None)
    refps_ctx = tc.tile_pool(name="refps", bufs=2, space="PSUM", side="right")
    refps = refps_ctx.__enter__()
    refine_all(refps)

    refps_ctx.__exit__(None, None, None)
    attps_ctx.__exit__(None, None, None)
    att_ctx.__exit__(None, None, None)
    o_ctx.__exit__(None, None, None)
    wr_ctx.__exit__(None, None, None)
    zf_ctx.__exit__(None, None, None)
    rp_ctx.__exit__(None, None, None)
    zb_ctx.__exit__(None, None, None)
    vp_ctx.__exit__(None, None, None)
    ep_ctx.__exit__(None, None, None)
    xbp_ctx.__exit__(None, None, None)


def _dedup_ldweights(nc):
    """Drop an InstLdweights whose weights AP is identical to the previous
    PE weight load with only InstMatmult in between on the PE stream.
    Safe: the array contents and the SBUF data under that AP cannot have
    legally changed before the dup load completes (any writer carries a
    WAR wait on it). Only waitless/updateless dups are dropped."""
    removed = 0
    for f in nc.m.functions:
        for blk in f.blocks:
            keep = []
            last_key = None
            for inst in blk.instructions:
                tn = type(inst).__name__
                eng = getattr(inst, "engine", None)
                if eng == mybir.EngineType.PE:
                    if tn == "InstLdweights":
                        si = inst.sync_info
                        clean = si is None or (
                            not si.on_wait and not si.on_update)
                        key = repr(inst.ins[0])
                        if clean and key == last_key:
                            removed += 1
                            continue
                        last_key = key
                    elif tn != "InstMatmult":
                        last_key = None
                keep.append(inst)
            if removed:
                del blk.instructions[:]
                for inst in keep:
                    blk.instructions.append(inst)
    return removed


def build(alpha: float, nrep: int = 1):
    nc = bacc.Bacc(
        "TRN2",
        target_bir_lowering=False,
        debug=False,
        enable_asserts=False,
        num_devices=NCORES,
    )

    io = {}
    io["xb_d"] = nc.dram_tensor("xb", [BL, C, HW], BF16, kind="ExternalInput")
    io["wk_d"] = nc.dram_tensor("wkt", [C, C], BF16, kind="ExternalInput")  # Wk.T
    io["wq_d"] = nc.dram_tensor("wqt", [C, C], BF16, kind="ExternalInput")
    io["wv_d"] = nc.dram_tensor("wvt", [C, C], BF16, kind="ExternalInput")
    io["wr_d"] = nc.dram_tensor("wrt", [C, C], BF16, kind="ExternalInput")
    for nm in ("bk", "bq"):
        io[nm] = nc.dram_tensor(nm, [1, C], BF16, kind="ExternalInput")
    io["bvc"] = nc.dram_tensor("bvc", [P, KC], F32, kind="ExternalInput")
    io["brc"] = nc.dram_tensor("brc", [P, KC], F32, kind="ExternalInput")
    io["ones_d"] = nc.dram_tensor("ones", [1, HW], BF16, kind="ExternalInput")
    io["out_d"] = nc.dram_tensor("out", [BL, C, HW], BF16, kind="ExternalOutput")

    io["s_in"] = [
        nc.dram_tensor(f"s_in{i}", [nd, P, C], BF16)
        for i, (_, nd) in enumerate(SCHUNKS)
    ]
    io["s_out"] = [
        nc.dram_tensor(f"s_out{i}", [nd, P, C], BF16, addr_space="Shared")
        for i, (_, nd) in enumerate(SCHUNKS)
    ]

    # const AP so ACT Exp can take bias=-SHIFT
    cshift = nc.alloc_sbuf_tensor("const-shift", [128, 1], F32)
    nc.gpsimd.memset(cshift.ap(), -SHIFT)
    nc.const_aps.aps[(F32, -SHIFT)] = cshift.ap()
    nc.all_engine_barrier()

    with tile.TileContext(nc) as tc:
        with tc.tile_pool(name="cpool", bufs=1, side="left") as cpool:
            ones = cpool.tile([1, HW], BF16, tag="ones")
            nc.sync.dma_start(ones[:], io["ones_d"].ap())
            bvc = cpool.tile([P, KC], F32, tag="bvc")
            nc.sync.dma_start(bvc[:], io["bvc"].ap())
            brc = cpool.tile([P, KC], F32, tag="brc")
            nc.sync.dma_start(brc[:], io["brc"].ap())
            io["ones_t"] = ones
            io["bvc_t"] = bvc
            io["brc_t"] = brc

            for _ in range(nrep):
                _emit(nc, tc, io, alpha)

    n = _dedup_ldweights(nc)
    import sys
    print(f"dedup_ldweights: removed {n}", file=sys.stderr)
    nc.compile()
    return nc


def make_in_maps(x, Wq, bq, Wk, bk, Wv, bv, Wr, br, alpha=0.1):
    bf = ml_dtypes.bfloat16
    alpha_f = float(np.asarray(alpha).reshape(-1)[0])
    xsb = np.asarray(x, dtype=np.float32).reshape(B, C, HW).astype(bf)
    w = {
        "wkt": np.ascontiguousarray(np.asarray(Wk, dtype=np.float32).T.astype(bf)),
        "wqt": np.ascontiguousarray(np.asarray(Wq, dtype=np.float32).T.astype(bf)),
        "wvt": np.ascontiguousarray(np.asarray(Wv, dtype=np.float32).T.astype(bf)),
        "wrt": np.ascontiguousarray(np.asarray(Wr, dtype=np.float32).T.astype(bf)),
    }
    rows = {
        "bk": np.asarray(bk, dtype=np.float32).reshape(1, C).astype(bf),
        "bq": np.asarray(bq, dtype=np.float32).reshape(1, C).astype(bf),
    }
    # per-partition bias columns: [P, KC]; chunk ct holds channels
    # ct*P..(ct+1)*P-1 in partition order
    bvc = np.ascontiguousarray(
        np.asarray(bv, dtype=np.float32).reshape(KC, P).T)
    brc = np.ascontiguousarray(
        (alpha_f * np.asarray(br, dtype=np.float32)).reshape(KC, P).T)
    in_maps = []
    for c in range(NCORES):
        in_maps.append({
            "xb": np.ascontiguousarray(xsb[c * BL:(c + 1) * BL]),
            **w,
            "ones": np.ones((1, HW), dtype=bf),
            "bvc": bvc,
            "brc": brc,
            **rows,
        })
    return in_maps


def kernel(x, Wq, bq, Wk, bk, Wv, bv, Wr, br, alpha):
    alpha_f = float(np.asarray(alpha).reshape(-1)[0])
    key = ("v7", alpha_f, AR_MODE, PHASES)
    if key not in _CACHE:
        _CACHE[key] = build(alpha_f)
    nc = _CACHE[key]

    in_maps = make_in_maps(x, Wq, bq, Wk, bk, Wv, bv, Wr, br, alpha_f)
    res = bass_utils.run_bass_kernel_spmd(nc, in_maps, core_ids=list(range(NCORES)))
    out = np.concatenate([res.results[c]["out"] for c in range(NCORES)], axis=0)
    return np.ascontiguousarray(out.reshape(B, C, S, S).astype(np.float32))


# revision 21
# speedup vs baseline: 1.1776x; 1.1776x over previous
"""ChannelAttention (Softmax2d-over-batch) Trainium2 kernel, 8-core SPMD.

v5: single fused pipeline; PE is kept continuously busy and the batch
AllReduce of S = sum_b exp(scores) is split into 3 chunks on separate
DRAM tensors, issued mid-stream so they land under the V GEMMs.

Structure (per core, 4 samples):
  kt:   Kt[b] = ((Wk @ x_b)^T + bk) -> SBUF bf16 [HW, C]    (PE 47us)
  qt0:  Qt cgroup0 (cols 0:512)                              (PE 14us)
  B:    for dt 0..9: scoresT[d,c] = Qt_dt^T Kt (both hwt)   (PE 43us)
          -> ACT exp -> E bf16; S[dt] = sum_b E_b[dt] (DVE tree)
        qt cgroups 1,2 interleaved into dt 0..5 so ACT exp
        (66us, the B bottleneck) overlaps PE qt work.
        AR chunks (gpsimd->TOPSP, Pool does nothing else):
          dt2 -> AR0(S[0:3]), dt5 -> AR1(S[3:6]), dt9 -> AR2(S[6:10])
  C1:   V GEMMs (PE 43us, N=512 over sample pairs) hide the ARs;
        per-dt: z=AR out -> f32 (ACT) -> 1/z (DVE) -> E*=R in place
        (DVE/GpSimd split), pipelined behind the AR chunk landings.
  C2:   att[b] = attnT-contract @ V[b] (PE 43us), psum->SBUF on ACT,
        att stored [P, ct, b, HW] so refine streams N=512.
  C3:   refine GEMMs (PE 43us) interleaved with C2 per sample-pair;
        out = alpha*psum + (alpha*br + x_bf16), stored per (b, ot).

SBUF plan (strict LIFO per side; ~204 KB/p peak):
  left:  cpool | xb 20K (whole kernel) | ktqt 40K (..B) | wk 25.6K (..kt)
         then sst 7.5K (B) | then wr 25.6K, o 3K, att 20K, wv 5K (C)
  right: (after kt) E 102.4K (B..att) | wq 25.6K (..dt5)
         then V 20K, zb 2.5K, r 5K (C)
"""

import os

import numpy as np
import ml_dtypes

import concourse.bass as bass
import concourse.tile as tile
from concourse import bacc, mybir
from concourse import bass_utils

B, C, S, HW = 32, 1280, 16, 256
P = 128
KC = C // P          # 10 chunks of the channel dim
NCORES = 8
BL = B // NCORES     # 4 samples per core
SHIFT = 45.0
CGROUPS = [(0, 512), (512, 512), (1024, 256)]  # psum-bank-sized col groups
F32 = mybir.dt.float32
BF16 = mybir.dt.bfloat16
AF = mybir.ActivationFunctionType

_CACHE = {}
# AllReduce chunking: list of (start_dt, n_dt); chunk issued after its
# last dt's S row is stored.
AR_MODE = os.environ.get("KERNEL_AR_MODE", "split3")
# phase truncation for differential timing: 1=kt 2=+qt0 3=+B 4=+V/recip 5=+att 6=full
PHASES = int(os.environ.get("KERNEL_PHASES", "6"))
SCHUNKS = {
    "split3": [(0, 3), (3, 3), (6, 4)],
    "split2": [(0, 5), (5, 5)],
    "single": [(0, 10)],
    "none": [(0, 3), (3, 3), (6, 4)],  # diagnostic: DMA copy, no collective
}[AR_MODE]


def _emit(nc, tc, io, alpha):
    ones, bvc, brc = io["ones_t"], io["bvc_t"], io["brc_t"]
    xb_d = io["xb_d"]
    wk_d, wq_d, wv_d, wr_d = io["wk_d"], io["wq_d"], io["wv_d"], io["wr_d"]
    out_d = io["out_d"]

    # ---------------- pools: left stack base ----------------
    xbp_ctx = tc.tile_pool(name="xbp", bufs=1, side="left")
    xbp = xbp_ctx.__enter__()
    xb = xbp.tile([P, KC, BL * HW], BF16, tag="xb")    # 20 KB/p, whole kernel
    kt_ctx = tc.tile_pool(name="ktp", bufs=1, side="left")
    ktp = kt_ctx.__enter__()
    kt = ktp.tile([P, 2, BL, C], BF16, tag="kt")       # 20 KB/p
    wk_ctx = tc.tile_pool(name="wkp", bufs=1, side="left")
    wkp = wk_ctx.__enter__()
    wk_sb = wkp.tile([P, KC, C], BF16, tag="wk")       # 25.6 KB/p
    brow_k = wkp.tile([1, C], BF16, tag="browk")
    # E + wq live on the right stack from the start so the wq load does
    # not land on (and wait for) the released wk zone.
    ep_ctx = tc.tile_pool(name="ep", bufs=1, side="right")
    ep = ep_ctx.__enter__()
    e_sb = ep.tile([P, BL, KC, C], BF16, tag="E")      # 102.4 KB/p
    wq_ctx = tc.tile_pool(name="wqp", bufs=1, side="right")
    wqp = wq_ctx.__enter__()
    wq_sb = wqp.tile([P, KC, C], BF16, tag="wq")       # 25.6 KB/p
    brow_q = wqp.tile([1, C], BF16, tag="browq")

    # load order on SP: wk, bias rows, xb per-sample (b0 first), wq
    nc.sync.dma_start(wk_sb[:], wk_d.ap().rearrange("(k p) n -> p k n", p=P))
    nc.sync.dma_start(brow_k[:], io["bk"].ap())
    for b in range(BL):
        nc.sync.dma_start(
            xb[:, :, b * HW:(b + 1) * HW],
            xb_d.ap()[b].rearrange("(k p) n -> p k n", p=P),
        )
    nc.sync.dma_start(wq_sb[:], wq_d.ap().rearrange("(k p) n -> p k n", p=P))
    nc.sync.dma_start(brow_q[:], io["bq"].ap())

    def proj_group(dest, w_sb, brow, cgs, cgl, b, hwt, psp):
        """dest[:, hwt, b, cgs:cgs+cgl] = (x_b^T W)[hw-chunk, cg] + bias."""
        ps = psp.tile([P, 512], F32, tag="psA")
        for k in range(KC):
            nc.tensor.matmul(
                ps[:, :cgl],
                xb[:, k, b * HW + hwt * P:b * HW + (hwt + 1) * P],
                w_sb[:, k, cgs:cgs + cgl],
                start=(k == 0),
                stop=False,
            )
        nc.tensor.matmul(
            ps[:, :cgl], ones[:, :P], brow[:, cgs:cgs + cgl],
            start=False, stop=True,
        )
        # DVE drain: keeps ACT free for the exp stream in B
        nc.vector.tensor_copy(dest[:, hwt, b, cgs:cgs + cgl], ps[:, :cgl])

    # ---------------- kt (whole) ----------------
    psA_ctx = tc.tile_pool(name="psA", bufs=2, space="PSUM", side="left")
    psA = psA_ctx.__enter__()
    for cgs, cgl in CGROUPS:
        for b in range(BL):
            for hwt in range(2):
                proj_group(kt, wk_sb, brow_k, cgs, cgl, b, hwt, psA)
    wk_ctx.__exit__(None, None, None)
    if PHASES <= 1:
        psA_ctx.__exit__(None, None, None)
        wq_ctx.__exit__(None, None, None)
        ep_ctx.__exit__(None, None, None)
        kt_ctx.__exit__(None, None, None)
        xbp_ctx.__exit__(None, None, None)
        return
    qt_ctx = tc.tile_pool(name="qtp", bufs=1, side="left")
    qtp = qt_ctx.__enter__()
    qt = qtp.tile([P, 2, BL, C], BF16, tag="qt")       # 20 KB/p

    # ---------------- qt cgroup0 ----------------
    cg0s, cg0l = CGROUPS[0]
    for b in range(BL):
        for hwt in range(2):
            proj_group(qt, wq_sb, brow_q, cg0s, cg0l, b, hwt, psA)

    if PHASES <= 2:
        psA_ctx.__exit__(None, None, None)
        qt_ctx.__exit__(None, None, None)
        wq_ctx.__exit__(None, None, None)
        ep_ctx.__exit__(None, None, None)
        kt_ctx.__exit__(None, None, None)
        xbp_ctx.__exit__(None, None, None)
        return

    # qt cgroups 1,2 remaining groups, interleaved into B's dt loop
    qt_tail = [
        (cgs, cgl, b, hwt)
        for cgs, cgl in CGROUPS[1:]
        for b in range(BL)
        for hwt in range(2)
    ]
    qt_per_dt = {0: 3, 1: 3, 2: 2, 3: 3, 4: 3, 5: 2}

    # which AR chunk owns dt, and the chunk-local row index
    dt2chunk = {}
    for ci, (d0, nd) in enumerate(SCHUNKS):
        for j in range(nd):
            dt2chunk[d0 + j] = (ci, j)

    # ---------------- fused B ----------------
    sst_ctx = tc.tile_pool(name="sst", bufs=1, side="left")
    sstp = sst_ctx.__enter__()                          # 12.5 KB/p
    # first two V-weight chunks prefetched during B so the V GEMMs can
    # start the instant the last scores matmul retires
    wv01 = sstp.tile([P, KC, 2 * P], BF16, tag="wv01")
    nc.sync.dma_start(
        wv01[:],
        wv_d.ap()[:, 0:2 * P].rearrange("(k p) n -> p k n", p=P),
    )
    psB_ctx = tc.tile_pool(name="psB", bufs=2, space="PSUM", side="right")
    psB = psB_ctx.__enter__()
    qi = 0
    for dt in range(KC):
        for b in range(BL):
            # one 2.5-bank psum tile; cg slices stay bank-aligned so each
            # matmul dest is within one bank; ONE exp per (dt,b) halves
            # the ACT instruction count in the ACT-bound B phase
            ps = psB.tile([P, C], F32, tag="psB")
            for cgs, cgl in CGROUPS:
                for hwt in range(2):
                    nc.tensor.matmul(
                        ps[:, cgs:cgs + cgl],
                        qt[:, hwt, b, dt * P:(dt + 1) * P],
                        kt[:, hwt, b, cgs:cgs + cgl],
                        start=(hwt == 0),
                        stop=(hwt == 1),
                    )
            nc.scalar.activation(
                e_sb[:, b, dt, :], ps[:], AF.Exp, bias=-SHIFT, scale=1.0,
            )
        # S[dt] = (E0+E1) + (E2+E3), bf16 pairwise tree on DVE
        s01 = sstp.tile([P, C], BF16, tag="s01")
        s23 = sstp.tile([P, C], BF16, tag="s23")
        st = sstp.tile([P, C], BF16, tag="st")
        nc.vector.tensor_add(s01[:], e_sb[:, 0, dt], e_sb[:, 1, dt])
        nc.vector.tensor_add(s23[:], e_sb[:, 2, dt], e_sb[:, 3, dt])
        nc.vector.tensor_add(st[:], s01[:], s23[:])
        ci, j = dt2chunk[dt]
        nc.sync.dma_start(io["s_in"][ci].ap()[j], st[:])
        # interleave remaining qt groups to keep PE ahead of ACT exp
        for _ in range(qt_per_dt.get(dt, 0)):
            cgs, cgl, b, hwt = qt_tail[qi]
            qi += 1
            proj_group(qt, wq_sb, brow_q, cgs, cgl, b, hwt, psA)
        if dt == 5:
            psA_ctx.__exit__(None, None, None)
            wq_ctx.__exit__(None, None, None)
        # AR chunk issues (gpsimd queue holds only the 3 collectives)
        for cix, (d0, nd) in enumerate(SCHUNKS):
            if dt == d0 + nd - 1:
                if AR_MODE == "none":
                    # diagnostic only: wrong result, measures non-AR path
                    nc.gpsimd.dma_start(
                        io["s_out"][cix].ap(), io["s_in"][cix].ap()
                    )
                else:
                    nc.gpsimd.collective_compute(
                        "AllReduce",
                        mybir.AluOpType.add,
                        replica_groups=[list(range(NCORES))],
                        ins=[io["s_in"][cix].ap()],
                        outs=[io["s_out"][cix].ap()],
                    )
    assert qi == len(qt_tail)
    if PHASES <= 3:
        psB_ctx.__exit__(None, None, None)
        sst_ctx.__exit__(None, None, None)
        qt_ctx.__exit__(None, None, None)
        kt_ctx.__exit__(None, None, None)
        ep_ctx.__exit__(None, None, None)
        xbp_ctx.__exit__(None, None, None)
        return

    # ---------------- C1: V GEMMs start immediately at B end ----------
    vp_ctx = tc.tile_pool(name="vp", bufs=1, side="right")
    vp = vp_ctx.__enter__()
    v_sb = vp.tile([P, KC, BL * HW], BF16, tag="V")     # 20 KB/p
    psV_ctx = tc.tile_pool(name="psV", bufs=2, space="PSUM", side="left")
    psV = psV_ctx.__enter__()

    # wv chunk loads (per vct) + wr load early on SP
    def recip_and_muls(dt):
        """z[dt] -> f32 -> 1/z -> E[:, :, dt, :] *= R  (in place)."""
        ci, j = dt2chunk[dt]
        zb = zbp.tile([P, C], BF16, tag="zb")
        # ACT-issued: keeps the AR-gated z loads out of SP's FIFO (the
        # wv/wr loads behind them must not wait on the collectives)
        nc.scalar.dma_start(zb[:], io["s_out"][ci].ap()[j])
        r = rp.tile([P, C], BF16, tag="r")
        for cgs, cgl in CGROUPS:
            # NR step may read only one non-scalar input from PSUM:
            # z lives in SBUF f32, scratch + result in PSUM.
            zf = zfp.tile([P, 512], F32, tag="zf")
            scr = psR.tile([P, 512], F32, tag="scr")
            rf = psR.tile([P, 512], F32, tag="rf")
            nc.scalar.copy(zf[:, :cgl], zb[:, cgs:cgs + cgl])
            nc.vector.reciprocal_approx_accurate(
                rf[:, :cgl], zf[:, :cgl], scr[:, :cgl]
            )
            nc.scalar.copy(r[:, cgs:cgs + cgl], rf[:, :cgl])
        for b in range(BL):
            eng = nc.gpsimd if b % 2 == 1 else nc.vector
            eng.tensor_mul(e_sb[:, b, dt], e_sb[:, b, dt], r[:])

    def v_gemms(vct, wvt, col=0):
        for bp in range(2):
            ps = psV.tile([P, 512], F32, tag="psV")
            for ci_ in range(KC):
                nc.tensor.matmul(
                    ps[:],
                    wvt[:, ci_, col:col + P],
                    xb[:, ci_, bp * 512:(bp + 1) * 512],
                    start=(ci_ == 0),
                    stop=(ci_ == KC - 1),
                )
            nc.vector.tensor_scalar_add(
                v_sb[:, vct, bp * 512:(bp + 1) * 512], ps[:],
                bvc[:, vct:vct + 1],
            )

    # vct 0,1 straight from the prefetched wv01 (B pools still open)
    v_gemms(0, wv01, col=0)
    v_gemms(1, wv01, col=P)
    psB_ctx.__exit__(None, None, None)
    sst_ctx.__exit__(None, None, None)
    qt_ctx.__exit__(None, None, None)
    kt_ctx.__exit__(None, None, None)

    # ---------------- remaining C pools ----------------
    zb_ctx = tc.tile_pool(name="zbp", bufs=1, side="right")
    zbp = zb_ctx.__enter__()                            # 2.5 KB/p
    rp_ctx = tc.tile_pool(name="rp", bufs=2, side="right")
    rp = rp_ctx.__enter__()                             # 5 KB/p
    zf_ctx = tc.tile_pool(name="zfp", bufs=2, side="right")
    zfp = zf_ctx.__enter__()                            # 4 KB/p
    wr_ctx = tc.tile_pool(name="wrp", bufs=1, side="left")
    wrp = wr_ctx.__enter__()
    wr_sb = wrp.tile([P, KC, C], BF16, tag="wr")        # 25.6 KB/p
    o_ctx = tc.tile_pool(name="op", bufs=2, side="left")
    op = o_ctx.__enter__()                              # 4 KB/p
    att_ctx = tc.tile_pool(name="attp", bufs=1, side="left")
    attp = att_ctx.__enter__()
    att_sb = attp.tile([P, KC, BL * HW], BF16, tag="att")  # 20 KB/p
    wv_ctx = tc.tile_pool(name="wvp", bufs=2, side="left")
    wvp = wv_ctx.__enter__()                            # 5 KB/p
    psR_ctx = tc.tile_pool(name="psR", bufs=1, space="PSUM", side="right")
    psR = psR_ctx.__enter__()

    def wv_load(vct):
        t = wvp.tile([P, KC, P], BF16, tag="wvc", name=f"wv{vct}")
        nc.sync.dma_start(
            t[:],
            wv_d.ap()[:, vct * P:(vct + 1) * P].rearrange(
                "(k p) n -> p k n", p=P),
        )
        return t

    # V GEMMs with recip/mul chains pipelined behind the AR landings
    recip_and_muls(0)
    recip_and_muls(1)
    wv_next = wv_load(2)
    wr_loaded = False
    for vct in range(2, KC):
        wvt = wv_next
        if vct < KC - 1:
            wv_next = wv_load(vct + 1)
        v_gemms(vct, wvt)
        if vct <= 5:
            recip_and_muls(vct)
        if vct == 3 and not wr_loaded:
            nc.sync.dma_start(
                wr_sb[:], wr_d.ap().rearrange("(k p) n -> p k n", p=P))
            wr_loaded = True
    for dt in range(6, KC):
        recip_and_muls(dt)
    wv_ctx.__exit__(None, None, None)
    psV_ctx.__exit__(None, None, None)
    if PHASES <= 4:
        psR_ctx.__exit__(None, None, None)
        att_ctx.__exit__(None, None, None)
        o_ctx.__exit__(None, None, None)
        wr_ctx.__exit__(None, None, None)
        zf_ctx.__exit__(None, None, None)
        rp_ctx.__exit__(None, None, None)
        zb_ctx.__exit__(None, None, None)
        vp_ctx.__exit__(None, None, None)
        ep_ctx.__exit__(None, None, None)
        xbp_ctx.__exit__(None, None, None)
        return

    # ---------------- C2/C3: att + refine, interleaved ----------------
    attps_ctx = tc.tile_pool(name="attps", bufs=3, space="PSUM", side="left")
    attps = attps_ctx.__enter__()

    def att_gemms(b):
        for ct in range(KC):
            ps = attps.tile([P, HW], F32, tag="psAtt")
            for dt in range(KC):
                nc.tensor.matmul(
                    ps[:],
                    e_sb[:, b, dt, ct * P:(ct + 1) * P],
                    v_sb[:, dt, b * HW:(b + 1) * HW],
                    start=(dt == 0),
                    stop=(dt == KC - 1),
                )
            nc.scalar.copy(att_sb[:, ct, b * HW:(b + 1) * HW], ps[:])

    out_ap = out_d.ap().rearrange("b (k p) n -> p k b n", p=P)

    def refine(bp, refps):
        o = None
        for ot in range(KC):
            ps = refps.tile([P, 512], F32, tag="psRef")
            for ct in range(KC):
                nc.tensor.matmul(
                    ps[:],
                    wr_sb[:, ct, ot * P:(ot + 1) * P],
                    att_sb[:, ct, bp * 512:(bp + 1) * 512],
                    start=(ct == 0),
                    stop=(ct == KC - 1),
                )
            if ot % 2 == 0:
                o = op.tile([P, 2, 2, HW], BF16, tag="o")
            for j in range(2):
                b = 2 * bp + j
                # out = alpha * psum + (alpha*br + x), bf16 store
                nc.vector.affine_then_add(
                    o[:, ot % 2, j, :], ps[:, j * HW:(j + 1) * HW],
                    xb[:, ot, b * HW:(b + 1) * HW],
                    scale=alpha, bias=brc[:, ot:ot + 1],
                )
            if ot % 2 == 1:
                for j in range(2):
                    nc.sync.dma_start(
                        out_ap[:, ot - 1:ot + 1, 2 * bp + j, :],
                        o[:, :, j, :],
                    )

    if PHASES <= 5:
        att_gemms(0)
        att_gemms(1)
        att_gemms(2)
        att_gemms(3)
        attps_ctx.__exit__(None, None, None)
        psR_ctx.__exit__(None, None, None)
        att_ctx.__exit__(None, None, None)
        o_ctx.__exit__(None, None, None)
        wr_ctx.__exit__(None, None, None)
        zf_ctx.__exit__(None, None, None)
        rp_ctx.__exit__(None, None, None)
        zb_ctx.__exit__(None, None, None)
        vp_ctx.__exit__(None, None, None)
        ep_ctx.__exit__(None, None, None)
        xbp_ctx.__exit__(None, None, None)
        return
    att_gemms(0)
    att_gemms(1)
    psR_ctx.__exit__(None, None, None)
    refps_ctx = tc.tile_pool(name="refps", bufs=2, space="PSUM", side="right")
    refps = refps_ctx.__enter__()
    refine(0, refps)
    att_gemms(2)
    att_gemms(3)
    refine(1, refps)

    refps_ctx.__exit__(None, None, None)
    attps_ctx.__exit__(None, None, None)
    att_ctx.__exit__(None, None, None)
    o_ctx.__exit__(None, None, None)
    wr_ctx.__exit__(None, None, None)
    zf_ctx.__exit__(None, None, None)
    rp_ctx.__exit__(None, None, None)
    zb_ctx.__exit__(None, None, None)
    vp_ctx.__exit__(None, None, None)
    ep_ctx.__exit__(None, None, None)
    xbp_ctx.__exit__(None, None, None)


def build(alpha: float, nrep: int = 1):
    nc = bacc.Bacc(
        "TRN2",
        target_bir_lowering=False,
        debug=False,
        enable_asserts=False,
        num_devices=NCORES,
    )

    io = {}
    io["xb_d"] = nc.dram_tensor("xb", [BL, C, HW], BF16, kind="ExternalInput")
    io["wk_d"] = nc.dram_tensor("wkt", [C, C], BF16, kind="ExternalInput")  # Wk.T
    io["wq_d"] = nc.dram_tensor("wqt", [C, C], BF16, kind="ExternalInput")
    io["wv_d"] = nc.dram_tensor("wvt", [C, C], BF16, kind="ExternalInput")
    io["wr_d"] = nc.dram_tensor("wrt", [C, C], BF16, kind="ExternalInput")
    for nm in ("bk", "bq"):
        io[nm] = nc.dram_tensor(nm, [1, C], BF16, kind="ExternalInput")
    io["bvc"] = nc.dram_tensor("bvc", [P, KC], F32, kind="ExternalInput")
    io["brc"] = nc.dram_tensor("brc", [P, KC], F32, kind="ExternalInput")
    io["ones_d"] = nc.dram_tensor("ones", [1, HW], BF16, kind="ExternalInput")
    io["out_d"] = nc.dram_tensor("out", [BL, C, HW], BF16, kind="ExternalOutput")

    io["s_in"] = [
        nc.dram_tensor(f"s_in{i}", [nd, P, C], BF16)
        for i, (_, nd) in enumerate(SCHUNKS)
    ]
    io["s_out"] = [
        nc.dram_tensor(f"s_out{i}", [nd, P, C], BF16, addr_space="Shared")
        for i, (_, nd) in enumerate(SCHUNKS)
    ]

    # const AP so ACT Exp can take bias=-SHIFT
    cshift = nc.alloc_sbuf_tensor("const-shift", [128, 1], F32)
    nc.gpsimd.memset(cshift.ap(), -SHIFT)
    nc.const_aps.aps[(F32, -SHIFT)] = cshift.ap()
    nc.all_engine_barrier()

    with tile.TileContext(nc) as tc:
        with tc.tile_pool(name="cpool", bufs=1, side="left") as cpool:
            ones = cpool.tile([1, HW], BF16, tag="ones")
            nc.sync.dma_start(ones[:], io["ones_d"].ap())
            bvc = cpool.tile([P, KC], F32, tag="bvc")
            nc.sync.dma_start(bvc[:], io["bvc"].ap())
            brc = cpool.tile([P, KC], F32, tag="brc")
            nc.sync.dma_start(brc[:], io["brc"].ap())
            io["ones_t"] = ones
            io["bvc_t"] = bvc
            io["brc_t"] = brc

            for _ in range(nrep):
                _emit(nc, tc, io, alpha)

    nc.compile()
    return nc


def make_in_maps(x, Wq, bq, Wk, bk, Wv, bv, Wr, br, alpha=0.1):
    bf = ml_dtypes.bfloat16
    alpha_f = float(np.asarray(alpha).reshape(-1)[0])
    xsb = np.asarray(x, dtype=np.float32).reshape(B, C, HW).astype(bf)
    w = {
        "wkt": np.ascontiguousarray(np.asarray(Wk, dtype=np.float32).T.astype(bf)),
        "wqt": np.ascontiguousarray(np.asarray(Wq, dtype=np.float32).T.astype(bf)),
        "wvt": np.ascontiguousarray(np.asarray(Wv, dtype=np.float32).T.astype(bf)),
        "wrt": np.ascontiguousarray(np.asarray(Wr, dtype=np.float32).T.astype(bf)),
    }
    rows = {
        "bk": np.asarray(bk, dtype=np.float32).reshape(1, C).astype(bf),
        "bq": np.asarray(bq, dtype=np.float32).reshape(1, C).astype(bf),
    }
    # per-partition bias columns: [P, KC]; chunk ct holds channels
    # ct*P..(ct+1)*P-1 in partition order
    bvc = np.ascontiguousarray(
        np.asarray(bv, dtype=np.float32).reshape(KC, P).T)
    brc = np.ascontiguousarray(
        (alpha_f * np.asarray(br, dtype=np.float32)).reshape(KC, P).T)
    in_maps = []
    for c in range(NCORES):
        in_maps.append({
            "xb": np.ascontiguousarray(xsb[c * BL:(c + 1) * BL]),
            **w,
            "ones": np.ones((1, HW), dtype=bf),
            "bvc": bvc,
            "brc": brc,
            **rows,
        })
    return in_maps


def kernel(x, Wq, bq, Wk, bk, Wv, bv, Wr, br, alpha):
    alpha_f = float(np.asarray(alpha).reshape(-1)[0])
    key = ("v8", alpha_f, AR_MODE, PHASES)
    if key not in _CACHE:
        _CACHE[key] = build(alpha_f)
    nc = _CACHE[key]

    in_maps = make_in_maps(x, Wq, bq, Wk, bk, Wv, bv, Wr, br, alpha_f)
    res = bass_utils.run_bass_kernel_spmd(nc, in_maps, core_ids=list(range(NCORES)))
    out = np.concatenate([res.results[c]["out"] for c in range(NCORES)], axis=0)
    return np.ascontiguousarray(out.reshape(B, C, S, S).astype(np.float32))
